# revision 19
# baseline (speedup 1.0000x reference)
"""Trainium2 Bass kernel for nn_AdditiveCouplingLayer.

y = x; y[:, 1::2] += MLP(x[:, 0::2])  with a 512->1024->1024->512 relu MLP.

Strategy: data-parallel over 8 NeuronCores (batch 65536 -> 8192/core),
weights replicated. The MLP's first two layers run in "transposed
activation" space (features on partitions, batch on the free dim) so
every matmul uses the natural weight layout; layer 3 swaps the matmul
operand roles (h2 slice stationary, W3 moving) so the translation comes
out in natural [batch, feature] layout — no output transpose needed.

All matmuls run in fp8 e4m3 with MatmulPerfMode.DoubleRow (2 PE rows
per cycle -> 2x the fp16 matmul throughput) and fp32 PSUM accumulation.
Weights are pre-scaled by 2048 on the host so their small entries
(std ~0.02) land in e4m3's normal range; the descale (exact 2^-11) is
folded into the scalar-engine activation for layers 1/2 and into the
DVE scalar_tensor_tensor for layer 3. b3 is pre-added into x's odd
columns on the host, so layer-3 assembly is a single fused
(psum * 1/s + x) DVE op. The output rel-err budget is dominated by x
itself (std 1) while the MLP translation is small (std ~0.1), so fp8's
~2% matmul error on the translation contributes only ~4e-3 overall.

DMA layout: HWDGE queues generate descriptors at ~10ns each, so
throughput is descriptor-size-bound. The host pre-permutes every load
into its exact SBUF layout ([128 partitions, free]) so each transfer is
one DMA with 2-16KB contiguous per-partition descriptors: mT tiles
(2KB), x tiles (16KB), whole weight matrices (4-8KB). Loads ride the
sync queue, activations own the scalar sequencer, DVE does the layer-3
adds, and the y stores ride the otherwise-idle gpsimd SWDGE queue.

The batch is cut into 15x512 + 2x256 tiles: the kernel tail after the
final matmul is the last tile's add+store chain, so the last tiles are
narrow, their adds run on DVE and Pool in parallel, and their full-row
stores (4KB descriptors) go to the sync/scalar queues which are idle by
then.
"""

import os
import sys

sys.path.insert(0, "/opt/trn_rl_repo")

import numpy as np

B, D, F, H = 65536, 1024, 512, 1024
NCORES = 8
BPC = B // NCORES  # rows per core
# Batch tile widths (sum = BPC). Narrow tiles at the START shorten the
# h1 activation chain that gates tile 0's layer 2 (L2 contracts over all
# of h1, so it waits for every h1 activation of the tile); narrow tiles
# at the END shorten the post-matmul add+store tail.
WIDTHS = [256, 256] + [512] * 14 + [256, 256]
WSCALE = 2048.0  # host-side weight pre-scale (power of 2: exact descale)

assert sum(WIDTHS) == BPC

_cache = {}


def _build():
    import concourse.bacc as bacc
    import concourse.tile as tile
    import concourse.mybir as mybir

    dt = mybir.dt
    AF = mybir.ActivationFunctionType
    DR = mybir.MatmulPerfMode.DoubleRow
    ALU = mybir.AluOpType

    nc = bacc.Bacc(
        "TRN2", target_bir_lowering=False, debug=False, num_devices=NCORES
    )

    NT = len(WIDTHS)
    r0s = [sum(WIDTHS[:t]) for t in range(NT)]  # tile start rows
    moffs = [4 * r for r in r0s]  # mT free-dim offsets ([128, 4*w] per tile)

    # All inputs pre-permuted on host into SBUF layout: [128, free].
    x_d = nc.dram_tensor(
        "x", [128, (BPC // 128) * D], dt.float32, kind="ExternalInput"
    ).ap()
    mT_d = nc.dram_tensor(
        "mT", [128, 4 * BPC], dt.float8e4, kind="ExternalInput"
    ).ap()
    w_d = {
        "w1": nc.dram_tensor("w1", [128, 4 * H], dt.float8e4, kind="ExternalInput").ap(),
        "w2": nc.dram_tensor("w2", [128, 8 * H], dt.float8e4, kind="ExternalInput").ap(),
        "w3": nc.dram_tensor("w3", [128, 8 * F], dt.float8e4, kind="ExternalInput").ap(),
    }
    b1_d = nc.dram_tensor("b1m", [128, H // 128], dt.float32, kind="ExternalInput").ap()
    b2_d = nc.dram_tensor("b2m", [128, H // 128], dt.float32, kind="ExternalInput").ap()
    y_d = nc.dram_tensor("y", [BPC, D], dt.float32, kind="ExternalOutput").ap()

    with tile.TileContext(nc) as tc:
        with (
            tc.tile_pool(name="wpool", bufs=1) as wpool,
            tc.tile_pool(name="xpool", bufs=3) as xpool,
            tc.tile_pool(name="mpool", bufs=3) as mpool,
            tc.tile_pool(name="hpool", bufs=3) as hpool,
            tc.tile_pool(name="pmm", bufs=6, space="PSUM") as pmm,
        ):
            # --- resident weights/biases ---
            def load_w(name, nk, cols, eng):
                """One contiguous DMA per weight matrix (host pre-permuted
                to the SBUF layout); returns the [128, nk, cols] view for
                DoubleRow pair slicing."""
                big = wpool.tile([128, nk * cols], dt.float8e4, tag=name, name=name)
                eng.dma_start(big[:], w_d[name][:])
                return big[:].rearrange("p (k c) -> p k c", k=nk)

            def load_b(name, ap, n):
                t = wpool.tile([128, n // 128], dt.float32, tag=name)
                nc.scalar.dma_start(t[:], ap[:])
                return t

            # PE warmup: junk matmuls on a zeroed scratch tile keep the PE
            # busy (and its power-state activity window open) while the
            # first real DMAs are in flight, so real matmuls start at full
            # clock with all PE quadrants active.
            scratch = wpool.tile([128, 512], dt.float16, tag="scratch")
            nc.gpsimd.memset(scratch[:], 0.0)
            pwarm = pmm.tile([128, 512], dt.float32, tag="warm", bufs=1)
            for _ in range(13):
                nc.tensor.matmul(
                    pwarm[:], scratch[:, :128], scratch[:], start=True, stop=True
                )

            # Startup: mT tile 0 on the sync queue and W1 on the gpsimd
            # (SWDGE) queue run in parallel, so the first real matmul's
            # operands are ready right as the warmup matmuls finish.
            # W2/W3 follow on the scalar queue behind the (tiny) bias
            # loads.
            w1r = load_w("w1", 4, H, nc.gpsimd)
            b1t = load_b("b1t", b1_d, H)
            b2t = load_b("b2t", b2_d, H)
            w2r = load_w("w2", 8, H, nc.scalar)
            w3r = load_w("w3", 8, F, nc.scalar)

            def layer(wr, nkp, ins_r, bt, oname, w):
                """Transposed-space fp8 layer: for each output 128-chunk m,
                out[:, m*w:] = fp8(relu(psum * 1/WSCALE + b))."""
                obig = hpool.tile([128, 8 * w], dt.float8e4, tag=oname, name=oname)
                for m in range(8):
                    p = pmm.tile([128, 512], dt.float32, tag="mm")
                    ms = slice(m * 128, (m + 1) * 128)
                    for kp in range(nkp):
                        nc.tensor.matmul(
                            p[:, :w],
                            wr[:, 2 * kp : 2 * kp + 2, ms],
                            ins_r[:, 2 * kp : 2 * kp + 2, :],
                            start=(kp == 0),
                            stop=(kp == nkp - 1),
                            perf_mode=DR,
                        )
                    nc.scalar.activation(
                        obig[:, m * w : (m + 1) * w],
                        p[:, :w],
                        AF.Relu,
                        bias=bt[:, m : m + 1],
                        scale=1.0 / WSCALE,
                    )
                return obig[:].rearrange("p (k c) -> p k c", k=8)

            def l1_tile(t):
                """mT load + layer 1 for one batch tile (issued one tile
                ahead of layers 2/3 so mT is naturally prefetched)."""
                w = WIDTHS[t]
                mbig = mpool.tile([128, 4 * w], dt.float8e4, tag="mbig", name="mbig")
                nc.sync.dma_start(mbig[:], mT_d[:, moffs[t] : moffs[t] + 4 * w])
                mr = mbig[:].rearrange("p (j c) -> p j c", j=4)
                return layer(w1r, 2, mr, b1t, "h1", w)

            h1 = l1_tile(0)
            for t in range(NT):
                w = WIDTHS[t]
                r0 = r0s[t]
                nch = w // 128  # 128-row chunks in this tile
                last = t == NT - 1

                h1_next = l1_tile(t + 1) if t + 1 < NT else None

                # x tile (natural layout per 128-row chunk, b3 pre-added to
                # odd cols on the host, pre-permuted so this is one DMA of
                # 16KB descriptors).
                xbig = xpool.tile([128, nch * D], dt.float32, tag="xbig")
                co = (r0 // 128) * D
                nc.sync.dma_start(xbig[:], x_d[:, co : co + nch * D])
                xb = [xbig[:, i * D : (i + 1) * D] for i in range(nch)]
                h2 = layer(w2r, 4, h1, b2t, "h2", w)

                # layer 3 in natural layout: stationary = h2 batch-slice
                # pair, moving = W3 pair  ->  psum[batch128, F]; then one
                # fused op per row-chunk: y_odd = psum * 1/WSCALE + x_odd,
                # with the store issued right behind it.
                for i in range(nch):
                    p = pmm.tile([128, 512], dt.float32, tag="mm")
                    bs = slice(i * 128, (i + 1) * 128)
                    for kp in range(4):
                        nc.tensor.matmul(
                            p[:],
                            h2[:, 2 * kp : 2 * kp + 2, bs],
                            w3r[:, 2 * kp : 2 * kp + 2, :],
                            start=(kp == 0),
                            stop=(kp == 3),
                            perf_mode=DR,
                        )
                    rows = y_d[r0 + i * 128 : r0 + (i + 1) * 128, :]
                    nc.vector.scalar_tensor_tensor(
                        xb[i][:, 1:D:2],
                        p[:],
                        1.0 / WSCALE,
                        xb[i][:, 1:D:2],
                        ALU.mult,
                        ALU.add,
                    )
                    if last:
                        # final tile: split each chunk's store into column
                        # halves on separate queues (full-partition APs —
                        # partition-sliced DMAs serialize onto one engine)
                        q01 = ((nc.sync, nc.gpsimd), (nc.scalar, nc.sync))[i % 2]
                        hw_ = D // 2
                        for h, qeng in enumerate(q01):
                            qeng.dma_start(
                                rows[:, h * hw_ : (h + 1) * hw_],
                                xb[i][:, h * hw_ : (h + 1) * hw_],
                            )
                    else:
                        nc.gpsimd.dma_start(rows[:], xb[i][:])

                if h1_next is not None:
                    h1 = h1_next

    nc.compile()
    return nc


MODE = "fp8"  # single mode; kept for test.py compatibility


def _get(mode=None):
    if "nc" not in _cache:
        _cache["nc"] = _build()
    return _cache["nc"]


def _in_maps(x, W1, b1, W2, b2, W3, b3):
    import ml_dtypes

    f8 = ml_dtypes.float8_e4m3

    def prep_w(w, nk, cols):
        # fp8-quantize (pre-scaled) and permute [nk*128, cols] into the
        # SBUF-resident layout [128, nk*cols]
        q = (np.asarray(w, np.float32) * WSCALE).astype(f8)
        return np.ascontiguousarray(
            q.reshape(nk, 128, cols).transpose(1, 0, 2).reshape(128, nk * cols)
        )

    common = dict(
        w1=prep_w(W1, 4, H),
        w2=prep_w(W2, 8, H),
        w3=prep_w(W3, 8, F),
        b1m=np.ascontiguousarray(np.asarray(b1, np.float32).reshape(-1, 128).T),
        b2m=np.ascontiguousarray(np.asarray(b2, np.float32).reshape(-1, 128).T),
    )
    x = np.asarray(x, np.float32)
    xb3 = np.array(x, np.float32, copy=True)  # b3 pre-added to odd columns
    xb3[:, 1::2] += np.asarray(b3, np.float32)
    in_maps = []
    for c in range(NCORES):
        sl = slice(c * BPC, (c + 1) * BPC)
        # x tiles in SBUF layout: [p, (chunk c)] = x[chunk*128 + p, c]
        xh = np.ascontiguousarray(
            xb3[sl]
            .reshape(BPC // 128, 128, D)
            .transpose(1, 0, 2)
            .reshape(128, (BPC // 128) * D)
        )
        # mT tiles in SBUF layout, per variable-width tile t:
        # [p, (t j c)] = x[r0_t + c, 2*(j*128+p)]
        mq = x[sl, 0::2].T.astype(f8)  # [F, BPC]
        blocks = []
        r0 = 0
        for w in WIDTHS:
            blk = mq[:, r0 : r0 + w].reshape(4, 128, w).transpose(1, 0, 2)
            blocks.append(blk.reshape(128, 4 * w))
            r0 += w
        mh = np.ascontiguousarray(np.concatenate(blocks, axis=1))
        in_maps.append(dict(common, x=xh, mT=mh))
    return in_maps


def kernel(x, W1, b1, W2, b2, W3, b3):
    from concourse.bass_utils import run_bass_kernel_spmd

    nc = _get()
    res = run_bass_kernel_spmd(
        nc, _in_maps(x, W1, b1, W2, b2, W3, b3), core_ids=list(range(NCORES))
    )
    return np.concatenate([res.results[c]["y"] for c in range(NCORES)], axis=0)


# revision 22
# speedup vs baseline: 1.0029x; 1.0029x over previous
"""Trainium2 Bass kernel for nn_AdditiveCouplingLayer.

y = x; y[:, 1::2] += MLP(x[:, 0::2])  with a 512->1024->1024->512 relu MLP.

Strategy: data-parallel over 8 NeuronCores (batch 65536 -> 8192/core),
weights replicated. The MLP's first two layers run in "transposed
activation" space (features on partitions, batch on the free dim) so
every matmul uses the natural weight layout; layer 3 swaps the matmul
operand roles (h2 slice stationary, W3 moving) so the translation comes
out in natural [batch, feature] layout — no output transpose needed.

All matmuls run in fp8 e4m3 with MatmulPerfMode.DoubleRow (2 PE rows
per cycle -> 2x the fp16 matmul throughput) and fp32 PSUM accumulation.
Weights are pre-scaled by 2048 on the host so their small entries
(std ~0.02) land in e4m3's normal range; the descale (exact 2^-11) is
folded into the scalar-engine activation for layers 1/2 and into the
DVE scalar_tensor_tensor for layer 3. b3 is pre-added into x's odd
columns on the host, so layer-3 assembly is a single fused
(psum * 1/s + x) DVE op. The output rel-err budget is dominated by x
itself (std 1) while the MLP translation is small (std ~0.1), so fp8's
~2% matmul error on the translation contributes only ~4e-3 overall.

DMA layout: HWDGE queues generate descriptors at ~10ns each, so
throughput is descriptor-size-bound. The host pre-permutes every load
into its exact SBUF layout ([128 partitions, free]) so each transfer is
one DMA with 2-16KB contiguous per-partition descriptors: mT tiles
(2KB), x tiles (16KB), whole weight matrices (4-8KB). Loads ride the
sync queue, activations own the scalar sequencer, DVE does the layer-3
adds, and the y stores ride the otherwise-idle gpsimd SWDGE queue.

The batch is cut into 15x512 + 2x256 tiles: the kernel tail after the
final matmul is the last tile's add+store chain, so the last tiles are
narrow and their stores are column-split across the sync/scalar/gpsimd
queues which are idle by then (full-partition APs — partition-sliced
DMAs serialize onto a single DMA engine).
"""

import sys

sys.path.insert(0, "/opt/trn_rl_repo")

import numpy as np

B, D, F, H = 65536, 1024, 512, 1024
NCORES = 8
BPC = B // NCORES  # rows per core
# Batch tile widths (sum = BPC). Narrow tiles at the END shorten the
# post-matmul add+store tail, which is the only work after the last
# matmul.
WIDTHS = [512] * 15 + [256, 256]
WSCALE = 2048.0  # host-side weight pre-scale (power of 2: exact descale)

assert sum(WIDTHS) == BPC

_cache = {}


def _build():
    import concourse.bacc as bacc
    import concourse.tile as tile
    import concourse.mybir as mybir

    dt = mybir.dt
    AF = mybir.ActivationFunctionType
    DR = mybir.MatmulPerfMode.DoubleRow
    ALU = mybir.AluOpType

    nc = bacc.Bacc(
        "TRN2", target_bir_lowering=False, debug=False, num_devices=NCORES
    )

    NT = len(WIDTHS)
    r0s = [sum(WIDTHS[:t]) for t in range(NT)]  # tile start rows
    moffs = [4 * r for r in r0s]  # mT free-dim offsets ([128, 4*w] per tile)

    # All inputs pre-permuted on host into SBUF layout: [128, free].
    x_d = nc.dram_tensor(
        "x", [128, (BPC // 128) * D], dt.float32, kind="ExternalInput"
    ).ap()
    mT_d = nc.dram_tensor(
        "mT", [128, 4 * BPC], dt.float8e4, kind="ExternalInput"
    ).ap()
    w_d = {
        "w1": nc.dram_tensor("w1", [128, 4 * H], dt.float8e4, kind="ExternalInput").ap(),
        "w2": nc.dram_tensor("w2", [128, 8 * H], dt.float8e4, kind="ExternalInput").ap(),
        "w3": nc.dram_tensor("w3", [128, 8 * F], dt.float8e4, kind="ExternalInput").ap(),
    }
    b1_d = nc.dram_tensor("b1m", [128, H // 128], dt.float32, kind="ExternalInput").ap()
    b2_d = nc.dram_tensor("b2m", [128, H // 128], dt.float32, kind="ExternalInput").ap()
    y_d = nc.dram_tensor("y", [BPC, D], dt.float32, kind="ExternalOutput").ap()

    with tile.TileContext(nc) as tc:
        with (
            tc.tile_pool(name="wpool", bufs=1) as wpool,
            tc.tile_pool(name="xpool", bufs=3) as xpool,
            tc.tile_pool(name="mpool", bufs=3) as mpool,
            tc.tile_pool(name="hpool", bufs=3) as hpool,
            tc.tile_pool(name="pmm", bufs=6, space="PSUM") as pmm,
        ):
            # --- resident weights/biases ---
            def load_w(name, nk, cols, eng):
                """One contiguous DMA per weight matrix (host pre-permuted
                to the SBUF layout); returns the [128, nk, cols] view for
                DoubleRow pair slicing."""
                big = wpool.tile([128, nk * cols], dt.float8e4, tag=name, name=name)
                eng.dma_start(big[:], w_d[name][:])
                return big[:].rearrange("p (k c) -> p k c", k=nk)

            def load_b(name, ap, n):
                t = wpool.tile([128, n // 128], dt.float32, tag=name)
                nc.scalar.dma_start(t[:], ap[:])
                return t

            # Startup: mT tile 0 on the sync queue and W1 on the gpsimd
            # (SWDGE) queue run in parallel, so the first real matmul's
            # operands are ready right as the warmup matmuls finish.
            # W1 is gpsimd's FIRST instruction (ahead of the warmup
            # memset) since its arrival gates the first real matmul.
            # W2/W3 follow on the scalar queue behind the (tiny) bias
            # loads.
            w1r = load_w("w1", 4, H, nc.gpsimd)

            # PE warmup: junk matmuls on a zeroed scratch tile keep the PE
            # busy (and its power-state activity window open) while the
            # first real DMAs are in flight, so real matmuls start at full
            # clock with all PE quadrants active.
            scratch = wpool.tile([128, 512], dt.float16, tag="scratch")
            nc.gpsimd.memset(scratch[:], 0.0)
            pwarm = pmm.tile([128, 512], dt.float32, tag="warm", bufs=1)
            for _ in range(13):
                nc.tensor.matmul(
                    pwarm[:], scratch[:, :128], scratch[:], start=True, stop=True
                )
            b1t = load_b("b1t", b1_d, H)
            b2t = load_b("b2t", b2_d, H)
            w2r = load_w("w2", 8, H, nc.scalar)
            w3r = load_w("w3", 8, F, nc.scalar)

            def layer(wr, nkp, ins_r, bt, oname, w):
                """Transposed-space fp8 layer: for each output 128-chunk m,
                out[:, m*w:] = fp8(relu(psum * 1/WSCALE + b))."""
                obig = hpool.tile([128, 8 * w], dt.float8e4, tag=oname, name=oname)
                for m in range(8):
                    p = pmm.tile([128, 512], dt.float32, tag="mm")
                    ms = slice(m * 128, (m + 1) * 128)
                    for kp in range(nkp):
                        nc.tensor.matmul(
                            p[:, :w],
                            wr[:, 2 * kp : 2 * kp + 2, ms],
                            ins_r[:, 2 * kp : 2 * kp + 2, :],
                            start=(kp == 0),
                            stop=(kp == nkp - 1),
                            perf_mode=DR,
                        )
                    nc.scalar.activation(
                        obig[:, m * w : (m + 1) * w],
                        p[:, :w],
                        AF.Relu,
                        bias=bt[:, m : m + 1],
                        scale=1.0 / WSCALE,
                    )
                return obig[:].rearrange("p (k c) -> p k c", k=8)

            def l1_tile(t):
                """mT load + layer 1 for one batch tile (issued one tile
                ahead of layers 2/3 so mT is naturally prefetched)."""
                w = WIDTHS[t]
                mbig = mpool.tile([128, 4 * w], dt.float8e4, tag="mbig", name="mbig")
                nc.sync.dma_start(mbig[:], mT_d[:, moffs[t] : moffs[t] + 4 * w])
                mr = mbig[:].rearrange("p (j c) -> p j c", j=4)
                return layer(w1r, 2, mr, b1t, "h1", w)

            h1 = l1_tile(0)
            for t in range(NT):
                w = WIDTHS[t]
                r0 = r0s[t]
                nch = w // 128  # 128-row chunks in this tile
                last = t == NT - 1

                h1_next = l1_tile(t + 1) if t + 1 < NT else None

                # x tile (natural layout per 128-row chunk, b3 pre-added to
                # odd cols on the host, pre-permuted so this is one DMA of
                # 16KB descriptors).
                xbig = xpool.tile([128, nch * D], dt.float32, tag="xbig")
                co = (r0 // 128) * D
                nc.sync.dma_start(xbig[:], x_d[:, co : co + nch * D])
                xb = [xbig[:, i * D : (i + 1) * D] for i in range(nch)]
                h2 = layer(w2r, 4, h1, b2t, "h2", w)

                # layer 3 in natural layout: stationary = h2 batch-slice
                # pair, moving = W3 pair  ->  psum[batch128, F]; then one
                # fused op per row-chunk: y_odd = psum * 1/WSCALE + x_odd,
                # with the store issued right behind it.
                for i in range(nch):
                    p = pmm.tile([128, 512], dt.float32, tag="mm")
                    bs = slice(i * 128, (i + 1) * 128)
                    for kp in range(4):
                        nc.tensor.matmul(
                            p[:],
                            h2[:, 2 * kp : 2 * kp + 2, bs],
                            w3r[:, 2 * kp : 2 * kp + 2, :],
                            start=(kp == 0),
                            stop=(kp == 3),
                            perf_mode=DR,
                        )
                    rows = y_d[r0 + i * 128 : r0 + (i + 1) * 128, :]
                    nc.vector.scalar_tensor_tensor(
                        xb[i][:, 1:D:2],
                        p[:],
                        1.0 / WSCALE,
                        xb[i][:, 1:D:2],
                        ALU.mult,
                        ALU.add,
                    )
                    if last:
                        # final tile: split each chunk's store into column
                        # halves on separate queues (full-partition APs —
                        # partition-sliced DMAs serialize onto one engine)
                        q01 = ((nc.sync, nc.gpsimd), (nc.scalar, nc.sync))[i % 2]
                        hw_ = D // 2
                        for h, qeng in enumerate(q01):
                            qeng.dma_start(
                                rows[:, h * hw_ : (h + 1) * hw_],
                                xb[i][:, h * hw_ : (h + 1) * hw_],
                            )
                    else:
                        nc.gpsimd.dma_start(rows[:], xb[i][:])

                if h1_next is not None:
                    h1 = h1_next

    nc.compile()
    return nc


MODE = "fp8"  # single mode; kept for test.py compatibility


def _get(mode=None):
    if "nc" not in _cache:
        _cache["nc"] = _build()
    return _cache["nc"]


def _in_maps(x, W1, b1, W2, b2, W3, b3):
    import ml_dtypes

    f8 = ml_dtypes.float8_e4m3

    def prep_w(w, nk, cols):
        # fp8-quantize (pre-scaled) and permute [nk*128, cols] into the
        # SBUF-resident layout [128, nk*cols]
        q = (np.asarray(w, np.float32) * WSCALE).astype(f8)
        return np.ascontiguousarray(
            q.reshape(nk, 128, cols).transpose(1, 0, 2).reshape(128, nk * cols)
        )

    common = dict(
        w1=prep_w(W1, 4, H),
        w2=prep_w(W2, 8, H),
        w3=prep_w(W3, 8, F),
        b1m=np.ascontiguousarray(np.asarray(b1, np.float32).reshape(-1, 128).T),
        b2m=np.ascontiguousarray(np.asarray(b2, np.float32).reshape(-1, 128).T),
    )
    x = np.asarray(x, np.float32)
    xb3 = np.array(x, np.float32, copy=True)  # b3 pre-added to odd columns
    xb3[:, 1::2] += np.asarray(b3, np.float32)
    in_maps = []
    for c in range(NCORES):
        sl = slice(c * BPC, (c + 1) * BPC)
        # x tiles in SBUF layout: [p, (chunk c)] = x[chunk*128 + p, c]
        xh = np.ascontiguousarray(
            xb3[sl]
            .reshape(BPC // 128, 128, D)
            .transpose(1, 0, 2)
            .reshape(128, (BPC // 128) * D)
        )
        # mT tiles in SBUF layout, per variable-width tile t:
        # [p, (t j c)] = x[r0_t + c, 2*(j*128+p)]
        mq = x[sl, 0::2].T.astype(f8)  # [F, BPC]
        blocks = []
        r0 = 0
        for w in WIDTHS:
            blk = mq[:, r0 : r0 + w].reshape(4, 128, w).transpose(1, 0, 2)
            blocks.append(blk.reshape(128, 4 * w))
            r0 += w
        mh = np.ascontiguousarray(np.concatenate(blocks, axis=1))
        in_maps.append(dict(common, x=xh, mT=mh))
    return in_maps


def kernel(x, W1, b1, W2, b2, W3, b3):
    from concourse.bass_utils import run_bass_kernel_spmd

    nc = _get()
    res = run_bass_kernel_spmd(
        nc, _in_maps(x, W1, b1, W2, b2, W3, b3), core_ids=list(range(NCORES))
    )
    return np.concatenate([res.results[c]["y"] for c in range(NCORES)], axis=0)
